# revision 29
# baseline (speedup 1.0000x reference)
"""Causal self-attention (GQA + RoPE) on 8 Trainium2 NeuronCores.

Sharding: global head-parallel. Core c owns heads [4c, 4c+4) — exactly one
KV head (c) — over both batches flattened to 4096 rows. Each core projects
q/k/v for its heads (full sequence, bf16 weights/activations into fp32
PSUM), runs causal attention in transposed layout (keys on partitions,
queries on free dim; softmax denominators via a ones-column appended to V).
Scores are processed in 256-key pairs (one Exp per pair) with the AV
matmuls software-pipelined one pair behind the score matmuls so the PE
never waits on the activation engine. Unnormalized y is staged per query
block straight into bf16 AllToAll payloads; query blocks run evens-then-
odds so the first AllToAll overlaps the remaining attention and the second
overlaps the first half of o_proj (whose normalization inputs are
prefetched mid-attention). Wo is prefetched in bf16 during attention; each
core emits the full o_proj for a 512-row output slice.
"""

import numpy as np
import ml_dtypes

import concourse.bass as bass
import concourse.mybir as mybir
import concourse.tile as tile
from concourse.bass_utils import run_bass_kernel_spmd

F32 = mybir.dt.float32
F32R = mybir.dt.float32r
BF16 = mybir.dt.bfloat16
AF = mybir.ActivationFunctionType

N_CORES = 8
B, T, C = 2, 2048, 2048
H, KV, D = 32, 8, 64
TT = B * T                     # 4096 flattened rows
HL = H // N_CORES              # 4 local heads
ROPE_THETA = 500000.0

N_TC = TT // 512               # 8 projection column chunks
N_QB = TT // 256               # 16 query blocks of 256
N_KC = TT // 128               # 32 key chunks of 128
VW = 68                        # padded per-chunk stride in the packed V tile

QB_ORDER = [0, 2, 4, 6, 8, 10, 12, 14, 1, 3, 5, 7, 9, 11, 13, 15]


def _split_waits(nc):
    """This container's walrus accepts at most ONE sync-wait per instruction.

    Move extra waits onto NoOp carriers immediately before the instruction on
    the same engine (engine executes in order, so blocking semantics hold)."""
    for f in nc.m.functions:
        for blk in f.blocks:
            insts = list(blk.instructions)
            out = []
            changed = False
            for inst in insts:
                si = inst.sync_info
                if si is not None and len(si.on_wait) > 1:
                    changed = True
                    waits = list(si.on_wait)
                    for w in waits[:-1]:
                        nop = mybir.InstNoOp(
                            name=nc.get_next_instruction_name(), ins=[], outs=[]
                        )
                        nop.engine = inst.engine
                        nop.sync_info = mybir.SyncInfo(on_wait=[w], on_update=[])
                        out.append(nop)
                    inst.sync_info = mybir.SyncInfo(
                        on_wait=[waits[-1]], on_update=list(si.on_update)
                    )
                out.append(inst)
            if changed:
                blk.instructions = out


def build_program():
    nc = bass.Bass("TRN2", target_bir_lowering=False, debug=False,
                   num_devices=N_CORES)

    XT = nc.dram_tensor("XT", [C, TT], BF16, kind="ExternalInput").ap()
    WQS = nc.dram_tensor("WQS", [C, HL * D], BF16, kind="ExternalInput").ap()
    WKV = nc.dram_tensor("WKV", [C, 2 * D], BF16, kind="ExternalInput").ap()
    WOP = nc.dram_tensor("WOP", [C, C], BF16, kind="ExternalInput").ap()
    COS = nc.dram_tensor("COS", [128, TT], F32, kind="ExternalInput").ap()
    SIN = nc.dram_tensor("SIN", [128, TT], F32, kind="ExternalInput").ap()
    MA = nc.dram_tensor("MA", [128, 256], BF16, kind="ExternalInput").ap()
    MB = nc.dram_tensor("MB", [128, 256], BF16, kind="ExternalInput").ap()
    ONEC = nc.dram_tensor("ONEC", [128, 1], BF16, kind="ExternalInput").ap()
    R2T = nc.dram_tensor("R2T", [128, 128], F32, kind="ExternalInput").ap()
    IDN = nc.dram_tensor("IDN", [128, 64], F32, kind="ExternalInput").ap()
    EALL = nc.dram_tensor("EALL", [32, 2048], F32, kind="ExternalInput").ap()
    OUTT = nc.dram_tensor("OUTT", [C, TT // N_CORES], F32,
                          kind="ExternalOutput").ap()

    # per-parity AllToAll payloads: rows 0:256 = 4 heads x 64 dims of
    # unnormalized y, rows 256:260 = softmax denominators; cols = 256 queries
    a2a_in = [nc.dram_tensor(f"a2a_in{p}", [N_CORES, HL * D + 4, 256],
                             BF16).ap() for p in range(2)]
    a2a_out = [nc.dram_tensor(f"a2a_out{p}", [N_CORES, HL * D + 4, 256],
                              BF16).ap() for p in range(2)]

    with tile.TileContext(nc) as tc, nc.allow_low_precision(reason="fp32r"):
        with tc.tile_pool(name="outer", bufs=1) as outer:
            onec = outer.tile([128, 1], BF16)
            ident = outer.tile([128, 64], F32R)
            eall = outer.tile([32, 2048], F32R)
            # Wo resident in bf16; DMAs issued during attention
            wo = [outer.tile([128, 1024], BF16, tag=f"wo{i}", name=f"wo{i}")
                  for i in range(32)]

            with tc.tile_pool(name="attn_state", bufs=1) as pers:
                cos = pers.tile([128, TT], F32R)
                sin = pers.tile([128, TT], F32R)
                r2t = pers.tile([128, 128], F32R)
                ma = pers.tile([128, 256], BF16)
                mb = pers.tile([128, 256], BF16)

                qTa = pers.tile([128, 2 * TT], F32R)
                kT = pers.tile([128, TT], F32R)
                vp = pers.tile([128, N_KC * VW], BF16)

                # ============== Phase P: projections + RoPE ==============
                with tc.tile_pool(name="wq_sb", bufs=1) as wq_pool:
                    wq_sb = [wq_pool.tile([128, HL * D], BF16,
                                          tag=f"wq{cb}", name=f"wq{cb}")
                             for cb in range(16)]
                    wkv_sb = [wq_pool.tile([128, 2 * D], BF16,
                                           tag=f"wkv{cb}", name=f"wkv{cb}")
                              for cb in range(16)]
                    # weights first: the first matmul only needs wq0/wkv0 +
                    # the first xt tile, so keep everything bigger behind them
                    for cb in range(16):
                        csl = slice(cb * 128, (cb + 1) * 128)
                        nc.scalar.dma_start(wq_sb[cb][:], WQS[csl, :])
                        nc.scalar.dma_start(wkv_sb[cb][:], WKV[csl, :])
                    # small tables needed by tcb0's rope/transpose next,
                    # then the big cos/sin tables, then attention-only data
                    nc.scalar.dma_start(r2t[:], R2T[:].bitcast(F32R))
                    nc.scalar.dma_start(onec[:], ONEC[:])
                    nc.scalar.dma_start(ident[:], IDN[:].bitcast(F32R))
                    nc.scalar.dma_start(cos[:, 0:2048],
                                        COS[:, 0:2048].bitcast(F32R))
                    nc.scalar.dma_start(sin[:, 0:2048],
                                        SIN[:, 0:2048].bitcast(F32R))
                    nc.scalar.dma_start(cos[:, 2048:TT],
                                        COS[:, 2048:TT].bitcast(F32R))
                    nc.scalar.dma_start(sin[:, 2048:TT],
                                        SIN[:, 2048:TT].bitcast(F32R))
                    nc.scalar.dma_start(ma[:], MA[:])
                    nc.scalar.dma_start(mb[:], MB[:])
                    nc.scalar.dma_start(eall[:], EALL[:].bitcast(F32R))

                    with tc.tile_pool(name="xt_sb", bufs=1) as xt_pool, \
                         tc.tile_pool(name="proj_ps", bufs=1, space="PSUM") as pps, \
                         tc.tile_pool(name="rot_ps", bufs=3, space="PSUM") as rps, \
                         tc.tile_pool(name="tp_ps", bufs=2, space="PSUM") as tps, \
                         tc.tile_pool(name="proj_tmp", bufs=2) as ptmp:
                        for tcb in range(N_TC):
                            tsl = slice(tcb * 512, (tcb + 1) * 512)
                            xt = [xt_pool.tile([128, 512], BF16,
                                               tag=f"xt{cb % 8}",
                                               name=f"xt{cb}")
                                  for cb in range(16)]
                            for cb in range(16):
                                eng = (nc.sync if tcb == 0 or cb % 2 == 0
                                       else nc.scalar)
                                eng.dma_start(
                                    xt[cb][:],
                                    XT[cb * 128:(cb + 1) * 128, tsl])
                            qp = [pps.tile([128, 512], F32, tag=f"qp{t}",
                                           name=f"qp{t}") for t in range(2)]
                            kvp = pps.tile([128, 512], F32, tag="kvp")
                            for cb in range(16):
                                st = dict(start=(cb == 0), stop=(cb == 15))
                                for t in range(2):
                                    nc.tensor.matmul(
                                        qp[t][:],
                                        wq_sb[cb][:, t * 128:(t + 1) * 128],
                                        xt[cb][:], **st)
                                nc.tensor.matmul(kvp[:], wkv_sb[cb][:],
                                                 xt[cb][:], **st)
                            # Drain all PSUM->SBUF copies first so the next
                            # tcb's matmuls (which reuse qp/kvp banks) and
                            # the PE rope/transpose work aren't serialized
                            # behind the rope arithmetic on the DVE.
                            qraw = [ptmp.tile([128, 512], F32R, tag="qraw",
                                              name=f"qraw{t}")
                                    for t in range(2)]
                            for t in range(2):
                                nc.vector.tensor_copy(qraw[t][:], qp[t][:])
                            vraw = ptmp.tile([128, 512], F32R, tag="vraw",
                                             bufs=1)
                            nc.vector.tensor_copy(vraw[64:128, :], kvp[64:128, :])
                            kraw = ptmp.tile([64, 512], F32R, tag="kraw",
                                             bufs=1)
                            nc.vector.tensor_copy(kraw[:], kvp[0:64, :])
                            # PE: rotation matmuls + V transposes
                            rot = [rps.tile([128, 512], F32, tag="rot",
                                            name=f"rot{t}")
                                   for t in range(2)]
                            for t in range(2):
                                nc.tensor.matmul(rot[t][:], r2t[:], qraw[t][:],
                                                 start=True, stop=True)
                            tpl = []
                            for r in range(4):
                                tp = tps.tile([128, 64], F32R, tag="tp")
                                nc.tensor.transpose(
                                    tp[:], vraw[64:128, r * 128:(r + 1) * 128],
                                    ident[64:128, :])
                                tpl.append(tp)
                            krot = rps.tile([64, 512], F32, tag="rot")
                            nc.tensor.matmul(krot[:], r2t[0:64, 0:64], kraw[:],
                                             start=True, stop=True)
                            # DVE: rope arithmetic + V stores
                            for r in range(4):
                                i = tcb * 4 + r
                                nc.vector.tensor_copy(
                                    vp[:, i * VW:i * VW + 64], tpl[r][:])
                                nc.vector.tensor_copy(
                                    vp[:, i * VW + 64:i * VW + 65], onec[:])
                            for t in range(2):
                                t1 = ptmp.tile([128, 512], F32R, tag="t1")
                                nc.vector.tensor_mul(t1[:], qraw[t][:],
                                                     cos[:, tsl])
                                t2 = ptmp.tile([128, 512], F32R, tag="t2")
                                nc.vector.tensor_mul(t2[:], rot[t][:],
                                                     sin[:, tsl])
                                nc.vector.tensor_add(qTa[:, t * TT + tcb * 512:t * TT + (tcb + 1) * 512], t1[:], t2[:])
                            k1 = ptmp.tile([64, 512], F32R, tag="k1", bufs=1)
                            nc.vector.tensor_mul(k1[:], kraw[:], cos[0:64, tsl])
                            k2 = ptmp.tile([64, 512], F32R, tag="k2", bufs=1)
                            nc.vector.tensor_mul(k2[:], krot[:], sin[0:64, tsl])
                            nc.vector.tensor_add(kT[0:64, tsl], k1[:], k2[:])
                            # mirror k to partitions 64:127 for row-packed scores
                            nc.sync.dma_start(kT[64:128, tsl], kT[0:64, tsl])

                # ===== Phase A: attention, evens then odds, direct staging ====
                # o_proj normalization state lives beside the attention pools
                # so piece 0's inputs can be prefetched mid-attention
                with tc.tile_pool(name="orhs", bufs=1) as orhs_pool, \
                     tc.tile_pool(name="otmp", bufs=1) as otmp:
                    orhs = [orhs_pool.tile([128, 256], BF16, tag=f"or{cc}",
                                           name=f"or{cc}")
                            for cc in range(16)]
                    raw = [orhs_pool.tile([128, 256], BF16, tag=f"raw{cc}",
                                          name=f"raw{cc}")
                           for cc in range(16)]

                    def emit_norm_pre(p, eng, cpeng=None):
                        """den + raw loads for piece p on a quiet engine."""
                        den_b = otmp.tile([32, 256], BF16, tag="denb", bufs=2)
                        for i in range(N_CORES):
                            eng.dma_start(
                                den_b[i * 4:(i + 1) * 4, :],
                                a2a_out[p][i, HL * D:HL * D + 4, :])
                        den = otmp.tile([32, 256], F32R, tag="den", bufs=2)
                        (cpeng or eng).tensor_copy(den[:], den_b[:])
                        for cc in range(16):
                            eng.dma_start(
                                raw[cc][:],
                                a2a_out[p][cc // 2,
                                           (cc % 2) * 128:((cc % 2) + 1) * 128,
                                           :])
                        return den

                    def emit_norm_fin(den):
                        rec = otmp.tile([32, 256], F32R, tag="rec", bufs=2)
                        nc.vector.reciprocal(rec[:], den[:])
                        return rec

                    dens = [None, None]

                    with tc.tile_pool(name="sc_ps", bufs=2, space="PSUM") as scps, \
                         tc.tile_pool(name="av_ps", bufs=2, space="PSUM") as avps, \
                         tc.tile_pool(name="ex_sb", bufs=3) as exp_pool, \
                         tc.tile_pool(name="stg_sb", bufs=6) as stg:
                        # prefetch Wo (bf16) while attention computes
                        for i in range(32):
                            half, cc = i // 16, i % 16
                            nc.sync.dma_start(
                                wo[i][:],
                                WOP[cc * 128:(cc + 1) * 128,
                                    half * 1024:(half + 1) * 1024])
                        for qb in QB_ORDER:
                            qsl = slice(qb * 256, (qb + 1) * 256)
                            wb = qb % 8
                            base_kc = (qb // 8) * 16
                            nch = 2 * wb + 2
                            y_lo = avps.tile([65, 512], F32, tag="ylo")
                            y_hi = avps.tile([65, 512], F32, tag="yhi")
                            qv = qTa[:].rearrange("p (t n) -> p t n",
                                                  t=2)[:, :, qsl]

                            def emit_av(ex, kc):
                                st = dict(start=(kc == base_kc),
                                          stop=(kc == base_kc + nch - 1))
                                vsl = vp[:, kc * VW:kc * VW + 65]
                                nc.tensor.matmul(y_lo[:], vsl,
                                                 ex[:, 0:512], **st)
                                nc.tensor.matmul(y_hi[:], vsl,
                                                 ex[:, 512:1024], **st)

                            prev = None
                            for ck in range(nch):
                                kc = base_kc + ck
                                ksl = slice(kc * 128, (kc + 1) * 128)
                                sc = scps.tile([128, 1024], F32, tag="sc")
                                nc.tensor.matmul(sc[:, 0:512],
                                                 kT[0:64, ksl], qv[0:64],
                                                 start=True, stop=True)
                                nc.tensor.matmul(sc[:, 512:1024],
                                                 kT[64:128, ksl], qv[64:128],
                                                 start=True, stop=True)
                                ex = exp_pool.tile([128, 1024], BF16, tag="ex")
                                nc.scalar.activation(ex[:], sc[:], AF.Exp,
                                                     scale=0.125)
                                if ck >= nch - 2:
                                    mk = ma if ck == nch - 2 else mb
                                    for blk in range(4):
                                        bsl = slice(blk * 256, (blk + 1) * 256)
                                        nc.vector.tensor_mul(
                                            ex[:, bsl], ex[:, bsl], mk[:])
                                if prev is not None:
                                    emit_av(*prev)
                                prev = (ex, kc)
                            emit_av(*prev)
                            # stage unnormalized y + denominators straight
                            # into the bf16 AllToAll payload for this parity
                            stgL = stg.tile([65, 512], BF16, tag="sl")
                            stgH = stg.tile([65, 512], BF16, tag="sh")
                            nc.vector.tensor_copy(stgL[:], y_lo[:])
                            nc.vector.tensor_copy(stgH[:], y_hi[:])
                            r, p = qb // 2, qb % 2
                            ap = a2a_in[p]
                            # one DMA per stage tile: partition q of the
                            # source maps to payload rows {q, 128+q}
                            # (t=0 / t=1 halves of the 512 free columns)
                            ydst = ap[r, 0:256, :].rearrange(
                                "(c q) n -> q c n", c=2)
                            ysrcL = stgL[0:64, :].rearrange(
                                "q (c n) -> q c n", c=2)
                            ysrcH = stgH[0:64, :].rearrange(
                                "q (c n) -> q c n", c=2)
                            nc.sync.dma_start(ydst[0:64], ysrcL)
                            nc.sync.dma_start(ydst[64:128], ysrcH)
                            for t in range(2):
                                csl = slice(t * 256, (t + 1) * 256)
                                nc.sync.dma_start(
                                    ap[r, 256 + t:257 + t, :], stgL[64:65, csl])
                                nc.sync.dma_start(
                                    ap[r, 258 + t:259 + t, :], stgH[64:65, csl])
                            if qb == 14:
                                # even-row payload complete: overlap its
                                # exchange with the odd query blocks
                                nc.gpsimd.collective_compute(
                                    "AllToAll", mybir.AluOpType.bypass,
                                    replica_groups=[list(range(N_CORES))],
                                    ins=[a2a_in[0][:]], outs=[a2a_out[0][:]])
                            if qb == 13:
                                # collective A landed long ago: prefetch
                                # piece 0's normalization inputs on the idle
                                # gpsimd engine (sync/scalar/DVE must not
                                # stall on the collective-A wait)
                                dens[0] = emit_norm_pre(0, nc.gpsimd)
                        nc.gpsimd.collective_compute(
                            "AllToAll", mybir.AluOpType.bypass,
                            replica_groups=[list(range(N_CORES))],
                            ins=[a2a_in[1][:]], outs=[a2a_out[1][:]])

                    # ================= Phase O: o_proj =================
                    # piece 0 only depends on the first AllToAll, so its
                    # matmuls overlap the second one
                    with tc.tile_pool(name="bc_ps", bufs=2, space="PSUM") as bcps, \
                         tc.tile_pool(name="o_ps", bufs=3, space="PSUM") as ops_pool, \
                         tc.tile_pool(name="oo_sb", bufs=3) as oo_pool:
                        for p in range(2):
                            if dens[p] is None:
                                dens[p] = emit_norm_pre(p, nc.scalar,
                                                        nc.vector)
                            rec = emit_norm_fin(dens[p])
                            for cc in range(16):
                                bc = bcps.tile([128, 256], F32, tag="bc")
                                nc.tensor.matmul(
                                    bc[:], eall[:, cc * 128:(cc + 1) * 128],
                                    rec[:], start=True, stop=True)
                                bcs = oo_pool.tile([128, 256], BF16, tag="bcs")
                                nc.vector.tensor_copy(bcs[:], bc[:])
                                nc.vector.tensor_mul(orhs[cc][:], raw[cc][:],
                                                     bcs[:])
                            for m in range(16):
                                half, mh = m // 8, m % 8
                                op = ops_pool.tile([128, 256], F32, tag="op")
                                for cc in range(16):
                                    nc.tensor.matmul(
                                        op[:],
                                        wo[half * 16 + cc][:,
                                                           mh * 128:(mh + 1) * 128],
                                        orhs[cc][:],
                                        start=(cc == 0), stop=(cc == 15))
                                osb = oo_pool.tile([128, 256], F32, tag="osb")
                                nc.vector.tensor_copy(osb[:], op[:])
                                nc.scalar.dma_start(
                                    OUTT[m * 128:(m + 1) * 128,
                                         p * 256:(p + 1) * 256], osb[:])

    _split_waits(nc)
    return nc


def host_inputs(x, Wq, Wk, Wv, Wo):
    """Per-core input maps (host-side sharding + precomputed tables)."""
    x = np.asarray(x, np.float32)
    Wq = np.asarray(Wq, np.float32)
    Wk = np.asarray(Wk, np.float32)
    Wv = np.asarray(Wv, np.float32)
    Wo = np.asarray(Wo, np.float32)
    bf = ml_dtypes.bfloat16

    xt = np.ascontiguousarray(x.reshape(TT, C).T).astype(bf)   # [C, TT]

    inv_freq = (1.0 / (ROPE_THETA ** (np.arange(0, D, 2) / D))).astype(np.float64)
    pos = (np.arange(TT) % T).astype(np.float64)
    ang = pos[None, :] * inv_freq[np.arange(128) % 32][:, None]   # [128, TT]
    cos_t = np.cos(ang).astype(np.float32)
    sin_t = np.sin(ang).astype(np.float32)

    ki = np.arange(128)[:, None]
    qf = np.arange(256)[None, :]
    ma = (ki <= qf).astype(bf)
    mb = (ki + 128 <= qf).astype(bf)

    R = np.zeros((64, 64), np.float32)
    for mrow in range(32):
        R[mrow, mrow + 32] = -1.0
        R[mrow + 32, mrow] = 1.0
    R2 = np.zeros((128, 128), np.float32)
    R2[0:64, 0:64] = R
    R2[64:128, 64:128] = R
    r2t = np.ascontiguousarray(R2.T)

    wop = np.empty((C, C), np.float32)
    row = 0
    for i in range(N_CORES):
        for t in range(2):
            for h in (4 * i + t, 4 * i + t + 2):
                wop[row:row + 64, :] = Wo[h * 64:(h + 1) * 64, :]
                row += 64
    wop = wop.astype(bf)

    eall = np.zeros((32, 2048), np.float32)
    for cc in range(16):
        i, t = cc // 2, cc % 2
        eall[i * 4 + t, cc * 128:cc * 128 + 64] = 1.0
        eall[i * 4 + t + 2, cc * 128 + 64:cc * 128 + 128] = 1.0

    maps = []
    for c in range(N_CORES):
        wqs = np.empty((C, HL * D), np.float32)
        col = 0
        for t in range(2):
            for h in (4 * c + t, 4 * c + t + 2):
                wqs[:, col:col + 64] = Wq[:, h * 64:(h + 1) * 64]
                col += 64
        wkv = np.concatenate(
            [Wk[:, c * 64:(c + 1) * 64], Wv[:, c * 64:(c + 1) * 64]], axis=1)
        maps.append({
            "XT": xt,
            "WQS": wqs.astype(bf),
            "WKV": np.ascontiguousarray(wkv).astype(bf),
            "WOP": wop,
            "COS": cos_t,
            "SIN": sin_t,
            "MA": ma,
            "MB": mb,
            "ONEC": np.ones((128, 1), bf),
            "R2T": r2t,
            "IDN": np.concatenate([np.zeros((64, 64), np.float32),
                                   np.eye(64, dtype=np.float32)], axis=0),
            "EALL": eall,
        })
    return maps


def assemble_output(results, dtype=np.float32):
    out = np.empty((TT, C), dtype)
    for c in range(N_CORES):
        out[c * 512:(c + 1) * 512, :] = results[c]["OUTT"].T
    return out.reshape(B, T, C)


_NC_CACHE = None


def get_program():
    global _NC_CACHE
    if _NC_CACHE is None:
        _NC_CACHE = build_program()
    return _NC_CACHE


def kernel(x, Wq, Wk, Wv, Wo):
    nc = get_program()
    maps = host_inputs(x, Wq, Wk, Wv, Wo)
    res = run_bass_kernel_spmd(nc, maps, list(range(N_CORES)))
    return assemble_output(res.results, np.asarray(x).dtype)


if __name__ == "__main__":
    rng = np.random.default_rng(0)
    s = 1.0 / np.sqrt(C)
    x = rng.standard_normal((B, T, C), dtype=np.float32)
    Wq = rng.standard_normal((C, C), dtype=np.float32) * s
    Wk = rng.standard_normal((C, KV * D), dtype=np.float32) * s
    Wv = rng.standard_normal((C, KV * D), dtype=np.float32) * s
    Wo = rng.standard_normal((C, C), dtype=np.float32) * s
    y = kernel(x=x, Wq=Wq, Wk=Wk, Wv=Wv, Wo=Wo)
    print("out", y.shape, y.dtype, float(np.abs(y).max()))
